# revision 1
# baseline (speedup 1.0000x reference)
"""Trainium2 Bass kernel for nn_Physics_Attention (sparse slice attention).

Contract: kernel(**inputs) takes the FULL unsharded inputs (as produced by
setup_inputs) and returns the FULL (4, 32768, 256) float32 output.

Sharding: 8 cores = 4 batches x 2 halves of the point dimension n.  Each core
processes one (batch, n-half) shard end-to-end; the slice-token pooling sums
are combined across the two cores of each batch with a pairwise AllReduce.

Key layout choices:
- Host pre-transposes x to (256, nloc) per core so the channel dim lands on
  partitions for the projections.
- The slice-logit projection is fused on the host: WXS = W_x @ blockdiag(W_slice),
  so logits = x @ WXS come out of one matmul chain (x_mid is never formed).
- Slice weights w are produced n-major (for pooling), normalized, cast fp16,
  then PE-transposed to a persistent g-major fp16 buffer (for the scatter).
- All big matmuls run float32r (full PE rate); the w/scatter path is fp16.
"""

import numpy as np

import concourse.bass as bass
import concourse.mybir as mybir
from concourse import bacc
from concourse.tile import TileContext
from concourse.bass_utils import run_bass_kernel_spmd

# Model dims (fixed by the problem).
B, N, C = 4, 32768, 256
H, D, G = 8, 64, 64
HD = H * D  # 512
HG = H * G  # 512
SCALE = D ** -0.5

NCORES = 8
NLOC = N // 2   # points per core
NT = 512        # moving-dim tile (columns per matmul)
NCH = 128       # contraction / partition chunk
PAIRS = H // 2  # head pairs
CCH = C // NCH  # 2 chunks of the input-channel dim
PCH = HD // NCH  # 4 chunks of the inner dim

F32 = mybir.dt.float32
F32R = mybir.dt.float32r
F16 = mybir.dt.float16
AF = mybir.ActivationFunctionType


def r(ap):
    """View a float32 AP as float32r (full-rate PE matmul mode)."""
    return ap.bitcast(F32R)


def build_nc(inv_temps, nloc=NLOC, bias_l_nz=False, b_fx_nz=False, b_out_nz=False):
    uniform_temp = bool(np.all(np.asarray(inv_temps) == inv_temps[0]))
    assert nloc % NT == 0
    jt_n = nloc // NT          # number of 512-wide n tiles
    jc_n = NT // NCH           # 128-chunks per tile (4)

    nc = bacc.Bacc()

    xT = nc.declare_dram_parameter("xT", [C, nloc], F32, isOutput=False)
    wxs = nc.declare_dram_parameter("wxs", [C, HG], F32, isOutput=False)
    w_fx = nc.declare_dram_parameter("w_fx", [C, HD], F32, isOutput=False)
    wq = nc.declare_dram_parameter("wq", [D, D], F32, isOutput=False)
    wk = nc.declare_dram_parameter("wk", [D, D], F32, isOutput=False)
    wv = nc.declare_dram_parameter("wv", [D, D], F32, isOutput=False)
    w_out = nc.declare_dram_parameter("w_out", [HD, C], F32, isOutput=False)
    ident_h = nc.declare_dram_parameter("ident_h", [NCH, NCH], F16, isOutput=False)
    ident_f = nc.declare_dram_parameter("ident_f", [NCH, NCH], F32, isOutput=False)
    if bias_l_nz:
        # (1, HG): per-head slice-logit bias, pre-temperature: b_x@W_slice + b_slice
        bsl_t = nc.declare_dram_parameter("bsl_t", [1, HG], F32, isOutput=False)
    if b_fx_nz:
        b_fx_in = nc.declare_dram_parameter("b_fx", [1, HD], F32, isOutput=False)
    if b_out_nz:
        b_out_in = nc.declare_dram_parameter("b_out", [1, C], F32, isOutput=False)
    y = nc.declare_dram_parameter("y", [nloc, C], F32, isOutput=True)

    groups = [[2 * i, 2 * i + 1] for i in range(B)]

    with TileContext(nc) as tc, \
         tc.tile_pool(name="persist", bufs=1) as pp:
        def ptile(shape, dtype, name):
            return pp.tile(shape, dtype, name=name, tag=name)

        wxs_sb = [ptile([NCH, HG], F16, f"wxs{ci}") for ci in range(CCH)]
        wfx_sb = [ptile([NCH, HD], F16, f"wfx{ci}") for ci in range(CCH)]
        wq_sb = ptile([D, D], F32R, "wq_sb")
        wk_sb = ptile([D, D], F32R, "wk_sb")
        wv_sb = ptile([D, D], F32R, "wv_sb")
        wout_sb = [ptile([NCH, C], F16, f"wout{pi}") for pi in range(PCH)]
        identh_sb = ptile([NCH, NCH], F16, "identh_sb")
        identf_sb = ptile([NCH, NCH], F32R, "identf_sb")
        # Persistent g-major slice weights, fp16: rows = 2 heads x 64 slices,
        # pair p occupies cols [p*nloc, (p+1)*nloc).
        wg_all = ptile([2 * G, PAIRS * nloc], F16, "wg_all")
        wg_v = wg_all[:].rearrange("a (p n) -> a p n", n=nloc)

        for ci in range(CCH):
            nc.gpsimd.dma_start(wxs_sb[ci][:], wxs[bass.ts(ci, NCH), :])
            nc.gpsimd.dma_start(wfx_sb[ci][:], w_fx[bass.ts(ci, NCH), :])
        nc.sync.dma_start(wq_sb[:], r(wq[:]))
        nc.sync.dma_start(wk_sb[:], r(wk[:]))
        nc.sync.dma_start(wv_sb[:], r(wv[:]))
        for pi in range(PCH):
            nc.gpsimd.dma_start(wout_sb[pi][:], w_out[bass.ts(pi, NCH), :])
        nc.sync.dma_start(identh_sb[:], ident_h[:])
        nc.sync.dma_start(identf_sb[:], r(ident_f[:]))
        if bias_l_nz:
            bsl_sb = ptile([1, HG], F16, "bsl_sb")
            nc.gpsimd.dma_start(bsl_sb[:], bsl_t[:])
            ones_sb = ptile([1, NCH], F16, "ones_sb")
            nc.vector.memset(ones_sb[:], 1.0)
        if b_fx_nz or b_out_nz:
            onesc_sb = ptile([1, NCH], F32R, "onesc_sb")
            nc.vector.memset(onesc_sb[:].bitcast(F32), 1.0)

        bfx_bc = bout_bc = None
        if b_fx_nz or b_out_nz:
            with tc.tile_pool(name="bias_ps", bufs=1, space="PSUM") as bias_ps_pool:
                if b_fx_nz:
                    bfxb_in = ptile([1, HD], F32R, "bfxb_in")
                    nc.sync.dma_start(bfxb_in[:], r(b_fx_in[:]))
                    ps = bias_ps_pool.tile([NCH, HD], F32, name="bfx_ps")
                    nc.tensor.matmul(ps[:], onesc_sb[:], bfxb_in[:], start=True, stop=True)
                    bfx_bc = ptile([NCH, HD], F32, "bfx_bc")
                    nc.vector.tensor_copy(bfx_bc[:], ps[:])
                if b_out_nz:
                    boutb_in = ptile([1, C], F32R, "boutb_in")
                    nc.sync.dma_start(boutb_in[:], r(b_out_in[:]))
                    ps = bias_ps_pool.tile([NCH, C], F32, name="bout_ps")
                    nc.tensor.matmul(ps[:], onesc_sb[:], boutb_in[:], start=True, stop=True)
                    bout_bc = ptile([NCH, C], F32, "bout_bc")
                    nc.vector.tensor_copy(bout_bc[:], ps[:])

        with tc.tile_pool(name="ar_dram", bufs=1, space="DRAM") as ar_pool:
            # DRAM bounce buffers for the pooled-token AllReduce.
            # Layout (G, H*65): head h -> cols h*65..h*65+63 pooled tokens,
            # col h*65+64 the norm (sum of slice weights).
            ar_in = ar_pool.tile([G, H * 65], F32, name="ar_in")
            ar_out = ar_pool.tile([G, H * 65], F32, name="ar_out")

            # ---- phase 1: projections, slice softmax, pooling ------------
            with tc.tile_pool(name="xt", bufs=4) as xt_pool, \
                 tc.tile_pool(name="fxp", bufs=6) as fxp_pool, \
                 tc.tile_pool(name="epool", bufs=3) as e_pool, \
                 tc.tile_pool(name="wpool", bufs=8) as w_pool, \
                 tc.tile_pool(name="dsm", bufs=8) as dsm_pool, \
                 tc.tile_pool(name="fx_ps", bufs=2, space="PSUM") as fx_ps, \
                 tc.tile_pool(name="lg_ps", bufs=2, space="PSUM") as lg_ps, \
                 tc.tile_pool(name="tr_ps", bufs=2, space="PSUM") as tr_ps, \
                 tc.tile_pool(name="st_ps", bufs=1, space="PSUM") as st_ps_pool:

                # Two PSUM banks hold the 8 heads' pooled (64x65) blocks:
                # head h=2p+hh -> tile hh, cols p*65..p*65+64 (partitions 0:64).
                st_ps = [
                    st_ps_pool.tile([G, PAIRS * 65], F32, name=f"st_ps{i}",
                                    tag=f"st_ps{i}")
                    for i in range(2)
                ]

                for jt in range(jt_n):
                    ns = jt * NT
                    xt = []
                    for ci in range(CCH):
                        t = xt_pool.tile([NCH, NT], F16, name="xt", tag=f"xt{ci}")
                        nc.gpsimd.dma_start(
                            t[:], xT[bass.ts(ci, NCH), bass.ds(ns, NT)]
                        )
                        xt.append(t)

                    # fx, n-major, packed (128n, 8*65) fp16 with ones cols.
                    fxp = []
                    for jc in range(jc_n):
                        ps = fx_ps.tile([NCH, HD], F32, name="fx_ps")
                        for ci in range(CCH):
                            nc.tensor.matmul(
                                ps[:],
                                xt[ci][:, bass.ts(jc, NCH)],
                                wfx_sb[ci][:],
                                start=(ci == 0),
                                stop=(ci == CCH - 1),
                            )
                        fxt = fxp_pool.tile([NCH, H * 65], F16, name="fxt",
                                            tag=f"fx{jc}")
                        src = ps[:].rearrange("p (h c) -> p h c", c=D)
                        dst = fxt[:].rearrange("p (h c) -> p h c", c=65)[:, :, 0:D]
                        if b_fx_nz:
                            nc.vector.tensor_tensor(
                                dst, src,
                                bfx_bc[:].rearrange("p (h c) -> p h c", c=D),
                                mybir.AluOpType.add,
                            )
                        else:
                            nc.vector.tensor_copy(dst, src)
                        ones_col = fxt[:].rearrange("p (h c) -> p h c", c=65)[:, :, D : D + 1]
                        nc.vector.memset(ones_col, 1.0)
                        fxp.append(fxt)

                    # fused slice logits (n-major, all heads), softmax,
                    # transpose to g-major, pooling.
                    for jc in range(jc_n):
                        lg = lg_ps.tile([NCH, HG], F32, name="lg")
                        for ci in range(CCH):
                            nc.tensor.matmul(
                                lg[:],
                                xt[ci][:, bass.ts(jc, NCH)],
                                wxs_sb[ci][:],
                                start=(ci == 0),
                                stop=(ci == CCH - 1) and not bias_l_nz,
                            )
                        if bias_l_nz:
                            nc.tensor.matmul(
                                lg[:], ones_sb[:], bsl_sb[:],
                                start=False, stop=True,
                            )
                        e_sb = e_pool.tile([NCH, HG], F32, name="e_sb")
                        dcol = dsm_pool.tile([NCH, 2 * H], F32, name="dcol")
                        if uniform_temp:
                            nc.scalar.activation(
                                e_sb[:], lg[:], AF.Exp,
                                scale=float(inv_temps[0]),
                            )
                        else:
                            for h in range(H):
                                nc.scalar.activation(
                                    e_sb[:, bass.ts(h, G)], lg[:, bass.ts(h, G)],
                                    AF.Exp, scale=float(inv_temps[h]),
                                )
                        nc.vector.reduce_sum(
                            dcol[:, 0:H],
                            e_sb[:].rearrange("a (h g) -> a h g", g=G),
                            axis=mybir.AxisListType.X,
                        )
                        nc.vector.reciprocal(dcol[:, H : 2 * H], dcol[:, 0:H])
                        w_sb = w_pool.tile([NCH, HG], F16, name="w_sb")
                        nc.vector.tensor_tensor(
                            w_sb[:].rearrange("a (h g) -> a h g", g=G),
                            e_sb[:].rearrange("a (h g) -> a h g", g=G),
                            dcol[:, H : 2 * H, None].to_broadcast((NCH, H, G)),
                            mybir.AluOpType.mult,
                        )
                        # transpose w -> g-major (4 pair blocks into one psum
                        # bank), persist fp16 with one strided copy; pool.
                        tr = tr_ps.tile([NCH, PAIRS * NCH], F16, name="tr")
                        for p in range(PAIRS):
                            nc.tensor.matmul(
                                tr[:, bass.ts(p, NCH)], w_sb[:, bass.ts(p, NCH)],
                                identh_sb[:], is_transpose=True,
                                start=True, stop=True, skip_group_check=True,
                            )
                        nc.vector.tensor_copy(
                            wg_v[:, :, bass.ds(ns + jc * NCH, NCH)],
                            tr[:].rearrange("a (p k) -> a p k", k=NCH),
                        )
                        for p in range(PAIRS):
                            for hh in range(2):
                                h = 2 * p + hh
                                first = (jt == 0 and jc == 0 and p == 0)
                                last = (jt == jt_n - 1 and jc == jc_n - 1
                                        and p == PAIRS - 1)
                                nc.tensor.matmul(
                                    st_ps[hh][:, p * 65 : (p + 1) * 65],
                                    w_sb[:, bass.ds(p * NCH + hh * G, G)],
                                    fxp[jc][:, h * 65 : (h + 1) * 65],
                                    start=first,
                                    stop=last,
                                    skip_group_check=True,
                                )

                # ---- AllReduce pooled tokens across the batch pair --------
                st_sb = ptile([G, H * 65], F32, "st_sb")
                for p in range(PAIRS):
                    for hh in range(2):
                        h = 2 * p + hh
                        nc.vector.tensor_copy(
                            st_sb[:, h * 65 : (h + 1) * 65],
                            st_ps[hh][:, p * 65 : (p + 1) * 65],
                        )
                nc.sync.dma_start(ar_in[:], st_sb[:])
                nc.gpsimd.collective_compute(
                    "AllReduce",
                    mybir.AluOpType.add,
                    ins=[ar_in[:]],
                    outs=[ar_out[:]],
                    replica_groups=groups,
                )
                sta_sb = ptile([G, H * 65], F32, "sta_sb")
                nc.sync.dma_start(sta_sb[:], ar_out[:])

        # ---- slice attention over (g=64) tokens, per head ----------------
        with tc.tile_pool(name="sa_sb", bufs=3) as sa_sb, \
             tc.tile_pool(name="sa_ps", bufs=4, space="PSUM") as sa_ps:
            osT_pair = [
                sa_sb.tile([NCH, D], F16, name=f"osT{p}", tag=f"osT{p}")
                for p in range(PAIRS)
            ]
            ow_sb = [
                sa_sb.tile([NCH, C], F16, name=f"ow{p}", tag=f"ow{p}")
                for p in range(PAIRS)
            ]
            nrm = sa_sb.tile([G, 2 * H], F32, name="nrm")
            tok = sa_sb.tile([G, H * D], F32R, name="tok")
            for h in range(H):
                nc.vector.tensor_scalar_add(
                    nrm[:, h : h + 1],
                    sta_sb[:, h * 65 + D : h * 65 + D + 1],
                    1e-5,
                )
                nc.vector.reciprocal(nrm[:, H + h : H + h + 1], nrm[:, h : h + 1])
                nc.vector.tensor_scalar_mul(
                    tok[:, bass.ts(h, D)],
                    sta_sb[:, h * 65 : h * 65 + D],
                    nrm[:, H + h : H + h + 1],
                )
            for h in range(H):
                p, hh = divmod(h, 2)
                ps_t = sa_ps.tile([D, D], F32R, name="sa_tr", tag="sa")
                nc.tensor.transpose(ps_t[:], tok[:, bass.ts(h, D)],
                                    identf_sb[:D, :D])
                tokT = sa_sb.tile([D, D], F32R, name="tokT", tag="tokT")
                nc.vector.tensor_copy(tokT[:], ps_t[:])
                ps_q = sa_ps.tile([D, D], F32, name="sa_q", tag="sa")
                nc.tensor.matmul(ps_q[:], wq_sb[:], tokT[:], start=True, stop=True)
                qT = sa_sb.tile([D, D], F32R, name="qT", tag="qT")
                nc.vector.tensor_copy(qT[:], ps_q[:])
                ps_k = sa_ps.tile([D, D], F32, name="sa_k", tag="sa")
                nc.tensor.matmul(ps_k[:], wk_sb[:], tokT[:], start=True, stop=True)
                kT = sa_sb.tile([D, D], F32R, name="kT", tag="kT")
                nc.vector.tensor_copy(kT[:], ps_k[:])
                ps_v = sa_ps.tile([D, D], F32, name="sa_v", tag="sa")
                nc.tensor.matmul(ps_v[:], tokT[:], wv_sb[:], start=True, stop=True)
                v = sa_sb.tile([D, D], F32R, name="v", tag="v")
                nc.vector.tensor_copy(v[:], ps_v[:])
                ps_s = sa_ps.tile([D, D], F32, name="sa_s", tag="sa")
                nc.tensor.matmul(ps_s[:], qT[:], kT[:], start=True, stop=True)
                ex = sa_sb.tile([D, D], F32, name="ex", tag="ex")
                dsum = sa_sb.tile([D, 2], F32, name="dsum", tag="dsum")
                nc.scalar.activation(
                    ex[:], ps_s[:], AF.Exp, scale=SCALE,
                    accum_out=dsum[:, 0:1],
                )
                nc.vector.reciprocal(dsum[:, 1:2], dsum[:, 0:1])
                attn = sa_sb.tile([D, D], F32R, name="attn", tag="attn")
                nc.vector.tensor_scalar_mul(attn[:], ex[:], dsum[:, 1:2])
                ps_at = sa_ps.tile([D, D], F32R, name="sa_at", tag="sa")
                nc.tensor.transpose(ps_at[:], attn[:], identf_sb[:D, :D])
                attnT = sa_sb.tile([D, D], F32R, name="attnT", tag="attnT")
                nc.vector.tensor_copy(attnT[:], ps_at[:])
                ps_os = sa_ps.tile([D, D], F32, name="sa_os", tag="sa")
                nc.tensor.matmul(ps_os[:], v[:], attnT[:], start=True, stop=True)
                nc.vector.tensor_copy(osT_pair[p][bass.ts(hh, G), :], ps_os[:])

            # OW[p] = [os_even @ W_out_even ; os_odd @ W_out_odd]  (128, C)
            for p in range(PAIRS):
                ps_ow = sa_ps.tile([NCH, C], F32, name="sa_ow", tag="sa",
                                   padded_shape=None)
                for hh in range(2):
                    nc.tensor.matmul(
                        ps_ow[bass.ts(hh, G), :],
                        osT_pair[p][bass.ts(hh, G), :],
                        wout_sb[p][bass.ts(hh, G), :],
                        start=True, stop=True,
                        tile_position=(hh * G, hh * G),
                    )
                nc.vector.tensor_copy(ow_sb[p][:], ps_ow[:])

            # ---- phase 2: fused scatter + output projection -------------
            with tc.tile_pool(name="ysb", bufs=3) as y_pool, \
                 tc.tile_pool(name="fin_ps", bufs=4, space="PSUM") as fin_ps:
                for jg in range(nloc // NCH):
                    ps = fin_ps.tile([NCH, C], F32, name="fin")
                    for p in range(PAIRS):
                        nc.tensor.matmul(
                            ps[:],
                            wg_v[:, p, bass.ds(jg * NCH, NCH)],
                            ow_sb[p][:],
                            start=(p == 0),
                            stop=(p == PAIRS - 1),
                        )
                    y_sb = y_pool.tile([NCH, C], F32, name="y_sb")
                    if b_out_nz:
                        nc.vector.tensor_tensor(
                            y_sb[:], ps[:], bout_bc[:], mybir.AluOpType.add
                        )
                    else:
                        nc.scalar.copy(y_sb[:], ps[:])
                    nc.sync.dma_start(
                        y[bass.ds(jg * NCH, NCH), :], y_sb[:]
                    )

    nc.finalize()
    return nc


def _prep_inputs(x, W_fx, b_fx, W_x, b_x, W_slice, b_slice, temperature,
                 Wq, Wk, Wv, W_out, b_out, nloc):
    f = np.float32
    temps = np.clip(np.asarray(temperature, f).reshape(H), 0.1, 5.0)
    inv_temps = (1.0 / temps).astype(f)
    Ws = np.asarray(W_slice, np.float64)
    b_slice = np.asarray(b_slice, np.float64).reshape(G)
    b_x64 = np.asarray(b_x, np.float64).reshape(HD)
    b_fx = np.asarray(b_fx, f).reshape(HD)
    b_fx_nz = bool(np.any(b_fx != 0))
    b_out = np.asarray(b_out, f).reshape(C)
    b_out_nz = bool(np.any(b_out != 0))

    # Fused slice-logit projection: logits = x @ WXS + bias_l (pre-temperature)
    Wx64 = np.asarray(W_x, np.float64).reshape(C, H, D)
    WXS = np.einsum("chd,dg->chg", Wx64, Ws).reshape(C, HG).astype(f)
    bias_l = (b_x64.reshape(H, D) @ Ws + b_slice[None, :]).reshape(HG).astype(f)
    bias_l_nz = bool(np.any(bias_l != 0))

    shared = {
        "wxs": np.ascontiguousarray(WXS),
        "w_fx": np.ascontiguousarray(np.asarray(W_fx, f)),
        "wq": np.ascontiguousarray(np.asarray(Wq, f)),
        "wk": np.ascontiguousarray(np.asarray(Wk, f)),
        "wv": np.ascontiguousarray(np.asarray(Wv, f)),
        "w_out": np.ascontiguousarray(np.asarray(W_out, f)),
        "ident_h": np.eye(NCH, dtype=np.float16),
        "ident_f": np.eye(NCH, dtype=f),
    }
    if bias_l_nz:
        shared["bsl_t"] = bias_l.reshape(1, HG)
    if b_fx_nz:
        shared["b_fx"] = b_fx.reshape(1, HD)
    if b_out_nz:
        shared["b_out"] = b_out.reshape(1, C)

    x = np.asarray(x, f)
    in_maps = []
    for core in range(NCORES):
        b, half = divmod(core, 2)
        xs = x[b, half * nloc : (half + 1) * nloc, :]
        m = dict(shared)
        m["xT"] = np.ascontiguousarray(xs.T)
        in_maps.append(m)
    return in_maps, inv_temps, bias_l_nz, b_fx_nz, b_out_nz


_NC_CACHE = {}


def get_nc_for(x, W_fx, b_fx, W_x, b_x, W_slice, b_slice, temperature,
               Wq, Wk, Wv, W_out, b_out):
    """Build (or fetch cached) program + per-core input maps for these inputs."""
    n = np.asarray(x).shape[1]
    nloc = n // 2
    in_maps, inv_temps, bl_nz, bf_nz, bo_nz = _prep_inputs(
        x, W_fx, b_fx, W_x, b_x, W_slice, b_slice, temperature,
        Wq, Wk, Wv, W_out, b_out, nloc,
    )
    key = (tuple(np.round(inv_temps, 9).tolist()), nloc, bl_nz, bf_nz, bo_nz)
    if key not in _NC_CACHE:
        _NC_CACHE[key] = build_nc(
            inv_temps, nloc=nloc, bias_l_nz=bl_nz, b_fx_nz=bf_nz, b_out_nz=bo_nz,
        )
    return _NC_CACHE[key], in_maps, nloc


def kernel(x, W_fx, b_fx, W_x, b_x, W_slice, b_slice, temperature,
           Wq, Wk, Wv, W_out, b_out, _trace=False, _trace_kwargs=None):
    x = np.asarray(x)
    b, n, c = x.shape
    assert (b, c) == (B, C) and n % (2 * NT) == 0, (b, n, c)
    nc, in_maps, nloc = get_nc_for(
        x, W_fx, b_fx, W_x, b_x, W_slice, b_slice, temperature,
        Wq, Wk, Wv, W_out, b_out,
    )
    res = run_bass_kernel_spmd(
        nc, in_maps, list(range(NCORES)), trace=_trace,
        **(_trace_kwargs or {}),
    )
    out = np.empty((B, n, C), np.float32)
    for core in range(NCORES):
        bb, half = divmod(core, 2)
        out[bb, half * nloc : (half + 1) * nloc, :] = res.results[core]["y"]
    if _trace:
        kernel._last_result = res
    return out

